# revision 5
# baseline (speedup 1.0000x reference)
"""Trainium2 Bass kernel for nn_Conv2d_mvm (PUMA bit-sliced crossbar conv emulation).

Math identity
-------------
The reference emulates an analog crossbar MVM: inputs become 16-bit
two's-complement bit-streams, weights become 2-bit slices of the 16-bit
magnitudes of their pos/neg parts, and ADC = clip(round(analog), 0, 511).
Each analog column sum is at most 128*3 = 384 < 511 and every quantity is a
small exact integer held in f32, so the ADC is the identity and the whole
pipeline is linear in the bits/slices. Shift-add therefore reconstructs

    out[p, c] = quant( (x_int[p, :] . w_int[c, :]) / 2^24 )

with x_int = round(patch * 2^12) (int16 wrap),
w_int = clip(round(relu(w)*2^12), 0, 65535) - clip(round(relu(-w)*2^12), 0, 65535),
quant(v) = clip(round(v * 2^12), -2^15, 2^15-1) / 2^12  (round-half-even).

Device kernel
-------------
Data-parallel over the P = 1024 output pixels: each of 8 cores computes 128
pixels (half of one batch image) against the replicated [L=576, Cout=128]
integer weight matrix, padded to 640 = 5 k-tiles of 128.

Exact fp16 split: x_int = 256*xh + xl with xh = x_int>>8 in [-128,127] and
xl = x_int&255 in [0,255] -- both exact in fp16 after cast.  The hi group
contracts against w_int (|w|<=2047, exact fp16); the lo group contracts
against wlo = w_int * 2^-8 (exponent shift, exact fp16, computed on-device
by one DVE op).  Then acc = (x.w)/256 = conv * 2^16, whose fp32->int16
RNE-saturating convert with scale 2^-4 IS the reference quantizer
(clip(round(conv*2^12), -2^15, 2^15-1)); the host rescales by 2^-12.
Products and partial sums stay well inside fp32's exact range apart from a
sub-ulp residue ~2^-5 of the quantizer step (verified: rel err 2.4e-6, and
identical to the all-fp16-shipped formulation).

Schedule (measured on HW via NTFF traces; exec ~12.7-13.0us vs 14.2us
baseline):
- The walrus/NEFF preamble (runtime start-kick ~3.3us + per-engine
  register TENSOR_LOADs + two all-engine barriers) pins every engine until
  ~5.8us; nothing can execute earlier, so the input DMA descriptor
  generations are HOISTED to the very front of the bass entry block
  (post-compile IR surgery) so they issue the moment each engine wakes.
- bacc's own entry all-engine barrier is REMOVED from the entry block: with
  hoisted SWDGE desc-gens it would stall every engine behind the gpsimd
  barrier-drain (which waits for SWDGE queue flush, ~2.6us); our program is
  fully semaphore-ordered and never touches the const APs the barrier
  protects.
- x ships bit-split as int8 (hi, signed) + uint8 (lo) on the gpsimd SWDGE
  queue with inline casts to fp16 (exact) -- 160KB HBM-side instead of
  320KB fp16.  w ships once as fp16 (160KB) on the scalar HWDGE queue
  (host-cast; values are integers <= 2047 so the cast is exact).
  Separate semaphores per DMA: a shared counter across different rings
  would release waiters out of order (verified failure mode).
- wlo = wt * 2^-8 is one DVE tensor_scalar over [128, 640] fp16 that runs
  in the shadow of the x transfers.
- 24 warm-up matmuls on an uninitialized scratch tile run from engine-wake
  (~6.0us) until just before the data lands (~9.1us), releasing the PE HAM
  clock-gate (1.2 -> 2.4 GHz after ~3.4us of sustained activity) for most
  of the real matmuls: measured 55.5ns warm vs 106.7ns cold cadence.
- Epilogue is ONE DVE op: tensor_scalar(out_i16, acc, 2^-4, +0) -- the
  fp32->int16 output conversion is round-to-nearest-even + saturating
  (verified on HW), which IS the reference quantizer.  The host rescales.
- The output store (32KB int16) issues on sync with a completion semaphore
  pinned to #205 but NO wait: the NEFF's multi-microsecond postamble
  (final all-engine barrier + per-engine semaphore-clear sweep) guarantees
  the data + sem-inc land long before the NEFF retires, and sem 205 is
  cleared by that sweep so re-execution is clean.
"""

import numpy as np

# Problem constants (hardcoded: kernel.py must be self-contained).
B, CIN, H, W = 4, 64, 16, 16
COUT = 128
K, PAD = 3, 1
IF = 12           # input frac bits
WF = 12           # weight frac bits
ACM_FRAC = 12
L = CIN * K * K   # 576
N_CORES = 8
ROWS_PER_CORE = H // 2            # 8 pixel rows per core
PIX_PER_CORE = ROWS_PER_CORE * W  # 128
KTW = 5                           # k-tiles (640 = 5*128, zero-padded)
KTX = 2 * KTW                     # fp16 x k-tiles: 5 hi + 5 lo
KT32 = 5                          # fp32 k-tiles (fallback path)
N_WARM = 27                       # PE warm-up matmuls during the input DMA

_CACHE = {}

_MAGIC = float(np.float32(1.5 * 2 ** 23))  # f32 RNE rounding constant
_INV_Q = 1.0 / (1 << ACM_FRAC)
_EPI = 1.0 / 16.0   # acc = conv * 2^16 -> int16 grid needs * 2^-4
_LO = float(-(1 << 15))
_HI = float((1 << 15) - 1)


def _hoist_to_front(nc, names):
    """Move the named instructions to the front of the entry block so their
    descriptor generation starts the moment each engine clears the NEFF
    preamble (the walrus preamble itself cannot be bypassed)."""
    main = nc.main_func.blocks[0]
    idx = next((i for i, ins in enumerate(main.instructions)
                if type(ins).__name__ in ('InstMemset', 'InstDrain')), 1)
    rest = [i for i in main.instructions[idx:]
            if getattr(i, 'name', None) in names]
    if rest:
        main.instructions[:] = [i for i in main.instructions if i not in rest]
    main.instructions[idx:idx] = rest


def _remove_entry_barrier(nc):
    """Drop bacc's entry all-engine barrier (Drain + EventSemaphore pairs)
    from the entry block.  Our program is fully semaphore-ordered; with
    hoisted SWDGE desc-gens the barrier would serialize everything behind
    the gpsimd queue-flush drain."""
    main = nc.main_func.blocks[0]
    drop = []
    for ins in main.instructions:
        tn = type(ins).__name__
        nm = getattr(ins, 'name', '') or ''
        si = getattr(ins, 'sync_info', None)
        uses_barrier_sem = False
        if si is not None:
            for x in list(si.on_wait) + list(si.on_update):
                if 'barrier' in (getattr(x, 'ant_name', '') or ''):
                    uses_barrier_sem = True
        if tn == 'InstEventSemaphore' and (nm.startswith('barrier_')
                                           or uses_barrier_sem):
            drop.append(ins)
        elif tn == 'InstDrain':
            drop.append(ins)
    main.instructions[:] = [i for i in main.instructions if i not in drop]


def _build_fp16_program():
    """Raw-Bass fp16 program: see module docstring for the schedule."""
    import concourse.bacc as bacc
    import concourse.mybir as mybir

    nc = bacc.Bacc("TRN2", target_bir_lowering=False, debug=False,
                   num_devices=N_CORES)
    xh8 = nc.dram_tensor("xh8", [128, KTW, PIX_PER_CORE], mybir.dt.int8,
                         kind="ExternalInput").ap()
    xl8 = nc.dram_tensor("xl8", [128, KTW, PIX_PER_CORE], mybir.dt.uint8,
                         kind="ExternalInput").ap()
    wk = nc.dram_tensor("wk", [128, KTW, COUT], mybir.dt.float16,
                        kind="ExternalInput").ap()
    out = nc.dram_tensor("out", [PIX_PER_CORE, COUT], mybir.dt.int16,
                         kind="ExternalOutput").ap()

    # Store-completion sem: never waited on; #205 is cleared near the END of
    # the NEFF postamble's per-engine sweep, ~4us after the inc can land.
    s_st = nc.alloc_semaphore("s_st", num=205)
    hoist_names = []
    with (
        nc.sbuf_tensor([128, KTX, PIX_PER_CORE], mybir.dt.float16) as xt,
        nc.sbuf_tensor([128, KTW, COUT], mybir.dt.float16) as wt,
        nc.sbuf_tensor([128, KTW, COUT], mybir.dt.float16) as wlo,
        nc.sbuf_tensor([128, 128], mybir.dt.float16) as warm,
        nc.sbuf_tensor([PIX_PER_CORE, COUT], mybir.dt.int16) as r16,
        nc.psum_tensor([PIX_PER_CORE, COUT], mybir.dt.float32) as acc,
        nc.psum_tensor([PIX_PER_CORE, COUT], mybir.dt.float32) as scratch,
        nc.semaphore("s_w") as s_w,
        nc.semaphore("s_xh") as s_xh,
        nc.semaphore("s_xl") as s_xl,
        nc.semaphore("s_mm") as s_mm,
        nc.semaphore("s_v") as s_v,
    ):
        # Input DMAs (desc-gens hoisted to the entry-block front).
        hoist_names.append(nc.scalar.dma_start(wt[:, :, :], wk[:, :, :])
                           .then_inc(s_w, 16).ins.name)
        # SWDGE inline casts: int8 -> fp16 and uint8 -> fp16 (exact).
        # single_packet compresses the packet stream so the SDMA engines
        # interleave less with the w queue on the binding xh path.
        hoist_names.append(nc.gpsimd.dma_start(xt[:, 0:KTW, :], xh8[:, :, :],
                                               single_packet=True)
                           .then_inc(s_xh, 16).ins.name)
        hoist_names.append(nc.gpsimd.dma_start(xt[:, KTW:KTX, :], xl8[:, :, :],
                                               single_packet=True)
                           .then_inc(s_xl, 16).ins.name)

        # wlo = wt * 2^-8 (exponent shift, exact in fp16).
        nc.vector.wait_ge(s_w, 16)
        nc.vector.tensor_scalar(wlo[:, :, :], wt[:, :, :], 1.0 / 256.0, 0.0,
                                op0=mybir.AluOpType.mult,
                                op1=mybir.AluOpType.add).then_inc(s_mm, 1)

        # Warm-ups on an uninitialized tile: results discarded, PE busy so
        # the HAM clock-gate can open before/during the real matmuls.
        for _i in range(N_WARM):
            nc.tensor.matmul(scratch[:, :], warm[:, :], warm[:, :],
                             start=True, stop=True)
        nc.tensor.wait_ge(s_w, 16)
        nc.tensor.wait_ge(s_xh, 16)
        for r in range(KTW):
            nc.tensor.matmul(acc[:, :], xt[:, r, :], wt[:, r, :],
                             start=(r == 0), stop=False)
        nc.tensor.wait_ge(s_xl, 16)
        nc.tensor.wait_ge(s_mm, 1)
        for r in range(KTW, KTX):
            mm = nc.tensor.matmul(acc[:, :], xt[:, r, :], wlo[:, r % KTW, :],
                                  start=False, stop=(r == KTX - 1))
        mm.then_inc(s_mm, 1)

        # fp32 -> int16 is RNE + saturating: exactly the reference
        # quantizer (clip(round(acc/2^4), -2^15, 2^15-1)).
        nc.vector.wait_ge(s_mm, 2)
        nc.vector.tensor_scalar(r16[:, :], acc[:, :], _EPI, 0.0,
                                op0=mybir.AluOpType.mult,
                                op1=mybir.AluOpType.add).then_inc(s_v, 1)

        nc.sync.wait_ge(s_v, 1)
        # single_packet aggregates the 256B store descriptors (the packet-
        # aggregation sweet spot) — measured marginally faster desc-gen/drain.
        nc.sync.dma_start(out[:, :], r16[:, :],
                          single_packet=True).then_inc(s_st, 16)

    _hoist_to_front(nc, set(hoist_names))
    _remove_entry_barrier(nc)
    nc.compile()
    return nc


def _build_fp32_program():
    """Fallback: 5 double-pumped fp32 matmuls over zero-padded k = 640."""
    import concourse.bacc as bacc
    import concourse.mybir as mybir
    import concourse.tile as tile

    nc = bacc.Bacc("TRN2", target_bir_lowering=False, debug=False,
                   num_devices=N_CORES)
    xk = nc.dram_tensor("xk", [128, KT32, PIX_PER_CORE], mybir.dt.float32,
                        kind="ExternalInput").ap()
    wk = nc.dram_tensor("wk", [128, KT32, COUT], mybir.dt.float32,
                        kind="ExternalInput").ap()
    out = nc.dram_tensor("out", [PIX_PER_CORE, COUT], mybir.dt.float32,
                         kind="ExternalOutput").ap()

    with tile.TileContext(nc) as tc:
        with (
            tc.tile_pool(name="sbuf", bufs=1) as pool,
            tc.tile_pool(name="psum", bufs=1, space="PSUM") as psum_pool,
        ):
            xt = pool.tile([128, KT32, PIX_PER_CORE], mybir.dt.float32, name="xt")
            wt = pool.tile([128, KT32, COUT], mybir.dt.float32, name="wt")
            nc.sync.dma_start(xt[:, :, :], xk[:, :, :])
            nc.gpsimd.dma_start(wt[:, :, :], wk[:, :, :])

            acc = psum_pool.tile([PIX_PER_CORE, COUT], mybir.dt.float32,
                                 name="acc")
            for r in range(KT32):
                nc.tensor.matmul(acc[:, :], xt[:, r, :], wt[:, r, :],
                                 start=(r == 0), stop=(r == KT32 - 1))
            res = pool.tile([PIX_PER_CORE, COUT], mybir.dt.float32, name="res")
            nc.vector.tensor_scalar(res[:, :], acc[:, :], _INV_Q, _MAGIC,
                                    op0=mybir.AluOpType.mult,
                                    op1=mybir.AluOpType.add)
            nc.vector.tensor_scalar(res[:, :], res[:, :], _MAGIC, _LO,
                                    op0=mybir.AluOpType.subtract,
                                    op1=mybir.AluOpType.max)
            nc.vector.tensor_scalar(res[:, :], res[:, :], _HI, _INV_Q,
                                    op0=mybir.AluOpType.min,
                                    op1=mybir.AluOpType.mult)
            nc.sync.dma_start(out[:, :], res[:, :])

    nc.compile()
    return nc


def _quantize_inputs(x, w):
    """Reproduce the reference's fixed-point quantization bit-exactly."""
    xi = np.round(x.astype(np.float32) * (1 << IF)).astype(np.int64)
    xi = ((xi + (1 << 15)) & 0xFFFF) - (1 << 15)  # int16 two's-complement wrap

    wf = w.reshape(COUT, L).astype(np.float32)
    w_pos = np.clip(np.round(np.clip(wf, 0, None) * (1 << WF)), 0, 65535)
    w_neg = np.clip(np.round(np.abs(np.clip(wf, None, 0)) * (1 << WF)), 0, 65535)
    wi = (w_pos - w_neg).astype(np.int64)  # [COUT, L], l = (cin, ki, kj)
    return xi, wi


def _im2col(xi):
    """[B, CIN, H, W] int -> patches [P, L] with l = (cin, ki, kj) order."""
    xpad = np.zeros((B, CIN, H + 2 * PAD, W + 2 * PAD), dtype=xi.dtype)
    xpad[:, :, PAD:PAD + H, PAD:PAD + W] = xi
    cols = [xpad[:, :, ki:ki + H, kj:kj + W]
            for ki in range(K) for kj in range(K)]
    p = np.stack(cols, axis=2)  # [B, CIN, K*K, H, W]
    return p.reshape(B, L, H * W).transpose(0, 2, 1).reshape(B * H * W, L)


def _prepare(x, w):
    """Quantize + stage inputs; returns (program_key, builder, in_maps)."""
    x = np.asarray(x, dtype=np.float32)
    w = np.asarray(w, dtype=np.float32)

    xi, wi = _quantize_inputs(x, w)          # int64: [B,CIN,H,W], [COUT, L]
    patches = _im2col(xi)                    # [P, L] int64
    wmat = wi.T                              # [L, COUT] int64

    # fp16 path is exact iff |w_int| fits fp16's 11-bit mantissa (the x split
    # halves xh in [-128,128) and xl in [0,256) are always fp16-exact).
    use_fp16 = np.abs(wi).max() <= 2047
    LP = KTW * 128

    if use_fp16:
        xh = (patches >> 8).astype(np.int8)      # arithmetic: [-128, 127]
        xl = (patches & 0xFF).astype(np.uint8)   # [0, 255]
        xhe = np.zeros((LP, B * H * W), dtype=np.int8)
        xle = np.zeros((LP, B * H * W), dtype=np.uint8)
        xhe[:L, :] = xh.T
        xle[:L, :] = xl.T
        we = np.zeros((LP, COUT), dtype=np.float16)
        we[:L, :] = wmat.astype(np.float16)      # exact: |w_int| <= 2047
        xht = np.ascontiguousarray(
            xhe.reshape(KTW, 128, B * H * W).transpose(1, 0, 2))
        xlt = np.ascontiguousarray(
            xle.reshape(KTW, 128, B * H * W).transpose(1, 0, 2))
        wtiles = np.ascontiguousarray(
            we.reshape(KTW, 128, COUT).transpose(1, 0, 2))
        key = "nc16"
        builder = _build_fp16_program
        in_maps = []
        for core in range(N_CORES):
            p0 = core * PIX_PER_CORE
            in_maps.append({
                "xh8": np.ascontiguousarray(xht[:, :, p0:p0 + PIX_PER_CORE]),
                "xl8": np.ascontiguousarray(xlt[:, :, p0:p0 + PIX_PER_CORE]),
                "wk": wtiles,
            })
    else:
        xe = np.zeros((KT32 * 128, B * H * W), dtype=np.float32)
        xe[:L, :] = patches.T.astype(np.float32)
        we = np.zeros((KT32 * 128, COUT), dtype=np.float32)
        we[:L, :] = wmat.astype(np.float32)
        xtiles = np.ascontiguousarray(
            xe.reshape(KT32, 128, B * H * W).transpose(1, 0, 2))
        wtiles = np.ascontiguousarray(
            we.reshape(KT32, 128, COUT).transpose(1, 0, 2))
        key = "nc32"
        builder = _build_fp32_program
        in_maps = []
        for core in range(N_CORES):
            p0 = core * PIX_PER_CORE
            in_maps.append({
                "xk": np.ascontiguousarray(xtiles[:, :, p0:p0 + PIX_PER_CORE]),
                "wk": wtiles,
            })
    return key, builder, in_maps


def kernel(x: np.ndarray, w: np.ndarray) -> np.ndarray:
    from concourse.bass_utils import run_bass_kernel_spmd

    key, builder, in_maps = _prepare(x, w)
    if key not in _CACHE:
        _CACHE[key] = builder()
    nc = _CACHE[key]

    results = run_bass_kernel_spmd(nc, in_maps, list(range(N_CORES))).results

    # Per-core shard: [128 pixels, COUT], pixels are (row, col) of half an image.
    out = np.empty((B, COUT, H, W), dtype=np.float32)
    for core in range(N_CORES):
        b, half = divmod(core, 2)
        r0 = half * ROWS_PER_CORE
        shard = results[core]["out"]
        if shard.dtype == np.int16:  # device returns the int16 quantizer grid
            shard = shard.astype(np.float32) * _INV_Q
        shard = shard.reshape(ROWS_PER_CORE, W, COUT)
        out[b, :, r0:r0 + ROWS_PER_CORE, :] = shard.transpose(2, 0, 1)
    return out
